# revision 11
# baseline (speedup 1.0000x reference)
"""Multi-head attention TRN2 kernel (B=4, S=2048, E=1024, H=16, D=64) on 8 cores.

Sharding: core c = (batch b = c//2, token-half hq = c%2). Each core receives
only its own 1024-token half of batch b's sequence (no rotation). K/V
projections are computed for the OWN half only and exchanged with the buddy
core via pairwise AllGather (replica groups {2b, 2b+1}), eliminating the
K/V-projection duplication. Keys stay in original order on both cores
(softmax over keys is order-invariant); queries are the own half, so each
core emits y rows for its own tokens.

Per-core dataflow:
  Phase A: x (f32) -> xs (bf16, converting DMA) -> xt via XBAR DMA transpose
    (no PE). V = x_own @ Wv + bv -> vloc -> AllGather -> vfull (chunked for
    overlap). K^T per pair for own tokens -> kloc -> AllGather -> kfull.
  Phase B: per head-pair p: Q^T JIT from own xt; kt loaded from kfull;
    scoresT [k, q] per k-tile as a row-tiled pair; exp split across scalar
    ACT (exact, exp(s/8 + ln lam)) and a custom DVE poly op; attnV with
    ones-augmented V stationary [k, 65] (row 64 = softmax denominator).
  Phase C: y = attn_outT.T @ W_out + b_out.
"""
from contextlib import ExitStack

import numpy as np

import concourse.bass as bass
import concourse.tile as tile
from concourse import bacc, mybir, dve_ops
from concourse.bass_utils import run_bass_kernel_spmd
from concourse.dve_spec import C0, C1, C2, Spec, Src0, sq
from concourse.tile_rust import add_dep_helper


def _ins(h):
    return getattr(h, "ins", h)


def _dep(after, before, why):
    add_dep_helper(_ins(after), _ins(before), reason=why)

F32R = mybir.dt.float32r
F32 = mybir.dt.float32
BF16 = mybir.dt.bfloat16
AF = mybir.ActivationFunctionType

B, S, E, H, D = 4, 2048, 1024, 16, 64
Q = 1024          # queries per core (own half)
SH = 1024         # own-half tokens for K/V projection
STO = 8           # own s-tiles
ET = 8            # e-tiles (contraction over E)
KT = 16           # k-tiles in attention (full S)
NP = 8            # head-pairs
N_CORES = 8
RG = [[0, 1], [2, 3], [4, 5], [6, 7]]

# lam*e^(s/8) ~ ((s - R)(s^2 + Bc*s + G))^4 for raw scores s in [-24, 24].
EXP_R = -5.36194375e+01
EXP_B = 4.77311991e+01
EXP_G = 3.76928874e+03
LN_LAM = 48.86652111696897   # scalar-ACT bias so both exp paths share lam
# k-tiles whose exp runs on the DVE custom op (rest: scalar ACT, exact)
DVE_KS = frozenset({1, 3, 5, 7, 9, 11, 13})


def _exp4_ref(in0, in1, s0, s1, imm2):
    pp = ((in0 - s0) * ((in0 * in0 + s1 * in0) + imm2)).astype(np.float32)
    y = (pp * pp).astype(np.float32)
    return (y * y).astype(np.float32)


def _register_exp4():
    for op in dve_ops.OPS:
        if op.name == "EXP4_ANT":
            return op
    x = Src0
    body = sq(sq((x - C0) * ((sq(x) + C1 * x) + C2)))
    op = dve_ops.DveOp(
        "EXP4_ANT",
        Spec(body=body, reference=_exp4_ref),
        subdim=False,
        uops_sha={"v3": "8097539a72e1c183"},
    )
    dve_ops.OPS.append(op)
    dve_ops.CUSTOM_DVE_SPECS[op.name] = op.spec
    dve_ops._SUB_OPCODE_FOR_NAME[op.name] = (
        dve_ops._CUSTOM_DVE_ROW_BASE + len(dve_ops.OPS) - 1
    )
    return op


EXP4 = _register_exp4()


def _bcast_dram(ap1d, n_part, n_free):
    """Broadcast a DRAM row across n_part partitions: [[0,n_part],[1,n_free]]."""
    return bass.AP(
        tensor=ap1d.tensor, offset=ap1d.offset, ap=[[0, n_part], [1, n_free]]
    )


def _emit(tc, nc, x, wqkv, bqkv, wout, bout, y,
          kloc, kfulls, vloc, vfulls, rscr, rscr2):
    with ExitStack() as ctx:
        xt_pool = ctx.enter_context(tc.tile_pool(name="xt", bufs=1))
        const = ctx.enter_context(tc.tile_pool(name="const", bufs=1))

        xt = xt_pool.tile([128, STO, ET, 128], BF16)

        wqb = const.tile([128, ET, E], BF16)
        wob = const.tile([128, ET, E], BF16)
        for vh in range(2):
            nc.gpsimd.dma_start(
                out=wqb[:, :, vh * 512 : (vh + 1) * 512],
                in_=wqkv[:, vh * 512 : (vh + 1) * 512].rearrange(
                    "(t p) n -> p t n", p=128),
            )
            nc.gpsimd.dma_start(
                out=wob[:, :, vh * 512 : (vh + 1) * 512],
                in_=wout[:, vh * 512 : (vh + 1) * 512].rearrange(
                    "(t p) n -> p t n", p=128),
            )
        bqk_t = const.tile([128, 24], F32)
        nc.gpsimd.dma_start(
            out=bqk_t, in_=bqkv.rearrange("(j p) -> p j", p=128).bitcast(F32)
        )
        lnlam_t = const.tile([128, 1], F32)
        nc.vector.memset(lnlam_t, LN_LAM)

        # ---- phase A: xs -> xt (PE transpose), V proj own half, K proj own ----
        with (
            tc.tile_pool(name="fconst", bufs=1) as fconst,
            tc.tile_pool(name="xload", bufs=4) as xload,
            tc.tile_pool(name="vps", bufs=2, space="PSUM") as vps,
            tc.tile_pool(name="tps", bufs=2, space="PSUM") as tps,
            tc.tile_pool(name="vev", bufs=4) as vev,
        ):
            from concourse.masks import make_identity
            ident_f = fconst.tile([128, 128], F32)
            make_identity(nc, ident_f)
            ident = fconst.tile([128, 128], BF16)
            nc.vector.tensor_copy(ident, ident_f)
            wvb = fconst.tile([128, ET, E], BF16)
            wkb = fconst.tile([128, ET, E], BF16)
            for vh in range(2):
                nc.gpsimd.dma_start(
                    out=wvb[:, :, vh * 512 : (vh + 1) * 512],
                    in_=wqkv[:, 2 * E + vh * 512 : 2 * E + (vh + 1) * 512].rearrange(
                        "(t p) n -> p t n", p=128),
                )
                nc.gpsimd.dma_start(
                    out=wkb[:, :, vh * 512 : (vh + 1) * 512],
                    in_=wqkv[:, E + vh * 512 : E + (vh + 1) * 512].rearrange(
                        "(t p) n -> p t n", p=128),
                )
            bv_t = fconst.tile([128, E], F32R)
            nc.gpsimd.dma_start(
                out=bv_t, in_=_bcast_dram(bqkv[2 * E : 2 * E + 1], 128, E)
            )

            def v_proj(st):
                ps = vps.tile([128, E], F32)
                for half in range(2):
                    for et in range(ET):
                        nc.tensor.matmul(
                            ps[:, half * 512 : (half + 1) * 512],
                            xt[:, st, et, :],
                            wvb[:, et, half * 512 : (half + 1) * 512],
                            start=(et == 0),
                            stop=(et == ET - 1),
                        )
                vb = vev.tile([128, E], BF16)
                nc.vector.tensor_add(vb, ps, bv_t)
                return nc.scalar.dma_start(
                    out=vloc[st * 128 : (st + 1) * 128, :], in_=vb)

            cc_v = [None] * 4
            vw = []
            for st in range(STO):
                xsf = xload.tile([128, E], F32R, tag="xsf")
                nc.sync.dma_start(out=xsf, in_=x[st * 128 : (st + 1) * 128, :])
                xs = xload.tile([128, E], BF16, tag="xs")
                nc.vector.tensor_copy(xs, xsf)
                for g in range(2):
                    ps = tps.tile([128, 4, 128], BF16)
                    for i in range(4):
                        e2 = g * 4 + i
                        nc.tensor.transpose(
                            ps[:, i, :], xs[:, e2 * 128 : (e2 + 1) * 128], ident
                        )
                    nc.vector.tensor_copy(xt[:, st, g * 4 : (g + 1) * 4, :], ps)
                vw.append(v_proj(st))
                if st % 2 == 1:
                    c = st // 2
                    cc = nc.gpsimd.collective_compute(
                        "AllGather",
                        mybir.AluOpType.bypass,
                        replica_groups=RG,
                        ins=[vloc[c * 256 : (c + 1) * 256, :]],
                        outs=[vfulls[c][:, :, :]],
                    )
                    for w in vw:
                        _dep(cc, w, "AG-V waits for vloc writes")
                    vw = []
                    cc_v[c] = cc

            def k_proj(p):
                ps = vps.tile([128, SH], F32)
                for half in range(2):
                    for et in range(ET):
                        nc.tensor.matmul(
                            ps[:, half * 512 : (half + 1) * 512],
                            wkb[:, et, p * 128 : (p + 1) * 128],
                            xt[:, half * 4 : (half + 1) * 4, et, :],
                            start=(et == 0),
                            stop=(et == ET - 1),
                        )
                kb = vev.tile([128, SH], BF16, tag="kb")
                nc.scalar.activation(
                    out=kb, in_=ps, func=AF.Identity,
                    bias=bqk_t[:, 8 + p : 9 + p], scale=1.0,
                )
                return nc.sync.dma_start(out=kloc[p], in_=kb)

            cc_k = [None] * 2
            kw = []
            for p in range(NP):
                kw.append(k_proj(p))
                if p % 4 == 3:
                    h4 = p // 4
                    cc = nc.gpsimd.collective_compute(
                        "AllGather",
                        mybir.AluOpType.bypass,
                        replica_groups=RG,
                        ins=[kloc[h4 * 4 : (h4 + 1) * 4]],
                        outs=[kfulls[h4][:, :, :, :]],
                    )
                    for w in kw:
                        _dep(cc, w, "AG-K waits for kloc writes")
                    kw = []
                    cc_k[h4] = cc

        ctx.cc_k, ctx.cc_v = cc_k, cc_v
        # ---- phase B: per-pair Q JIT + attention, software-pipelined ----
        aout_pool = ctx.enter_context(tc.tile_pool(name="aout", bufs=1))
        aout = aout_pool.tile([128, NP, Q], BF16)

        with (
            tc.tile_pool(name="qt", bufs=2) as qtp,
            tc.tile_pool(name="kt", bufs=2) as ktp,
            tc.tile_pool(name="vp", bufs=2) as vpp,
            tc.tile_pool(name="pt", bufs=4) as ptp,
            tc.tile_pool(name="ev", bufs=2) as evp,
            tc.tile_pool(name="qkps", bufs=1, space="PSUM") as qkps,
            tc.tile_pool(name="scps", bufs=2, space="PSUM") as scps,
            tc.tile_pool(name="accps", bufs=2, space="PSUM") as accps,
        ):

            def build_pair(p):
                """Allocate pair-p input tiles; return (tiles, emission thunks)."""
                qt_t = qtp.tile([128, Q], BF16)
                kt_t = ktp.tile([128, S], BF16)
                vp = vpp.tile([128, KT, 2, 65], BF16)
                th = []
                def kt_load(r, p=p, kt_t=kt_t):
                    d = nc.sync.dma_start(
                        out=kt_t[:, r * 1024 : (r + 1) * 1024],
                        in_=kfulls[p // 4][r, p % 4],
                    )
                    _dep(d, ctx.cc_k[p // 4], "kt load waits on AG-K")
                for r in range(2):
                    th.append(lambda r=r: kt_load(r))

                def vp_load(h, r, c, p=p, vp=vp):
                    d = nc.sync.dma_start(
                        out=vp[:, r * 8 + c * 2 : r * 8 + c * 2 + 2, h, 0:64],
                        in_=vfulls[c][
                            r, :, p * 128 + h * 64 : p * 128 + h * 64 + 64
                        ].rearrange("(t p2) d -> p2 t d", p2=128),
                    )
                    _dep(d, ctx.cc_v[c], "vp load waits on AG-V")
                for h in range(2):
                    for r in range(2):
                        for c in range(4):
                            th.append(lambda h=h, r=r, c=c: vp_load(h, r, c))
                th.append(lambda: nc.vector.memset(vp[:, :, :, 64:65], 1.0))

                # Q^T for own 1024 queries
                ps_box = []

                def alloc():
                    qk_ps = qkps.tile([128, 1024], F32, name="qk_ps", tag="qk")
                    ps_box.append(qk_ps)
                th.append(alloc)
                for half in range(2):
                    for et in range(ET):
                        th.append(lambda half=half, et=et: nc.tensor.matmul(
                            ps_box[0][:, half * 512 : (half + 1) * 512],
                            wqb[:, et, p * 128 : (p + 1) * 128],
                            xt[:, half * 4 : (half + 1) * 4, et, :],
                            start=(et == 0),
                            stop=(et == ET - 1),
                        ))
                th.append(lambda: nc.scalar.activation(
                    out=qt_t, in_=ps_box[0], func=AF.Identity,
                    bias=bqk_t[:, p : p + 1], scale=1.0,
                ))
                return {"qt": qt_t, "kt": kt_t, "vp": vp}, th

            cur, th0 = build_pair(0)
            for t in th0:
                t()
            ev_pending = []

            for p in range(NP):
                if p + 1 < NP:
                    nxt, pending = build_pair(p + 1)
                else:
                    nxt = None
                    pending = []
                pending = list(pending)
                qt_t, kt_t, vp = cur["qt"], cur["kt"], cur["vp"]
                for qh in range(2):
                    qsl = slice(qh * 512, (qh + 1) * 512)
                    acc0 = accps.tile([128, 512], F32, tag="acc")
                    acc1 = accps.tile([128, 512], F32, tag="acc")
                    pts = [None] * KT
                    for k in range(KT):
                        sc = scps.tile([128, 1024], F32, tag="sc")
                        nc.tensor.matmul(
                            sc[:, 0:512],
                            kt_t[0:64, k * 128 : (k + 1) * 128],
                            qt_t[0:64, qsl],
                            start=True, stop=True,
                        )
                        nc.tensor.matmul(
                            sc[:, 512:1024],
                            kt_t[64:128, k * 128 : (k + 1) * 128],
                            qt_t[64:128, qsl],
                            start=True, stop=True,
                        )
                        if k >= 1:
                            pt_p = pts[k - 1]
                            nc.tensor.matmul(
                                acc0[0:65, :], vp[:, k - 1, 0, :], pt_p[:, 0:512],
                                start=(k - 1 == 0), stop=(k - 1 == KT - 1),
                            )
                            nc.tensor.matmul(
                                acc1[0:65, :], vp[:, k - 1, 1, :], pt_p[:, 512:1024],
                                start=(k - 1 == 0), stop=(k - 1 == KT - 1),
                            )
                        pt_t = ptp.tile([128, 1024], BF16)
                        pts[k] = pt_t
                        if k in DVE_KS:
                            nc.vector._custom_dve(
                                EXP4, out=pt_t[:, :], in0=sc,
                                s0=EXP_R, s1=EXP_B, imm2=EXP_G,
                            )
                        else:
                            nc.scalar.activation(
                                out=pt_t, in_=sc, func=AF.Exp,
                                scale=0.125, bias=lnlam_t[:, 0:1],
                            )
                        for _ in range(2):
                            if pending:
                                pending.pop(0)()
                        if k in (4, 6) and ev_pending:
                            ev_pending.pop(0)()
                    nc.tensor.matmul(
                        acc0[0:65, :], vp[:, KT - 1, 0, :], pts[KT - 1][:, 0:512],
                        start=False, stop=True,
                    )
                    nc.tensor.matmul(
                        acc1[0:65, :], vp[:, KT - 1, 1, :], pts[KT - 1][:, 512:1024],
                        start=False, stop=True,
                    )
                    # eviction: fast psum release on scalar; the DVE pieces
                    # of the reciprocal dance are deferred into the next
                    # unit's loop so they never head-of-line block the exps.
                    ridx = p * 2 + qh
                    au0 = evp.tile([128, 512], F32, tag="au0")
                    nc.scalar.copy(au0[0:65, :], acc0[0:65, :])
                    au1 = evp.tile([128, 512], F32, tag="au1")
                    nc.scalar.copy(au1[0:65, :], acc1[0:65, :])
                    nc.gpsimd.dma_start(out=rscr[ridx : ridx + 1, 0:512], in_=au0[64:65, :])
                    nc.gpsimd.dma_start(out=rscr[ridx : ridx + 1, 512:1024], in_=au1[64:65, :])
                    rw = evp.tile([64, 16], F32, tag="rw")
                    nc.gpsimd.dma_start(
                        out=rw, in_=rscr[ridx : ridx + 1, :].rearrange("o (p f) -> (o p) f", p=64)
                    )
                    rwr = evp.tile([64, 16], F32, tag="rwr")

                    def dance1(rw=rw, rwr=rwr, ridx=ridx):
                        nc.vector.reciprocal(rwr, rw)
                        nc.gpsimd.dma_start(
                            out=rscr2[ridx : ridx + 1, :].rearrange("o (p f) -> (o p) f", p=64),
                            in_=rwr,
                        )

                    def dance2(au0=au0, au1=au1, ridx=ridx, p=p, qsl=qsl):
                        sc0 = evp.tile([64, 512], F32, name="sc0", tag="sc0")
                        nc.gpsimd.dma_start(out=sc0, in_=_bcast_dram(rscr2[ridx, 0:1], 64, 512))
                        sc1 = evp.tile([64, 512], F32, name="sc1", tag="sc1")
                        nc.gpsimd.dma_start(out=sc1, in_=_bcast_dram(rscr2[ridx, 512:513], 64, 512))
                        nc.vector.tensor_mul(aout[0:64, p, qsl], au0[0:64, :], sc0)
                        tmp1 = evp.tile([64, 512], BF16, name="tmp1", tag="tmp1")
                        nc.vector.tensor_mul(tmp1, au1[0:64, :], sc1)
                        nc.gpsimd.dma_start(out=aout[64:128, p, qsl], in_=tmp1)

                    ev_pending.append(dance1)
                    ev_pending.append(dance2)
                for t in pending:
                    t()
                cur = nxt
            for t in ev_pending:
                t()
            ev_pending.clear()

        # ---- phase C: y = attn_out @ W_out + b_out ----
        with (
            tc.tile_pool(name="yps", bufs=4, space="PSUM") as yps,
            tc.tile_pool(name="yev", bufs=3) as yev,
        ):
            bout_t = yev.tile([128, E], F32, tag="bout")
            nc.gpsimd.dma_start(out=bout_t, in_=_bcast_dram(bout[0:1], 128, E))
            for half in range(2):
                for qt_i in range(8):
                    ps = yps.tile([128, 512], F32)
                    for p8 in range(8):
                        nc.tensor.matmul(
                            ps,
                            aout[:, p8, qt_i * 128 : (qt_i + 1) * 128],
                            wob[:, p8, half * 512 : (half + 1) * 512],
                            start=(p8 == 0),
                            stop=(p8 == 7),
                        )
                    yb = yev.tile([128, 512], F32)
                    nc.vector.tensor_add(
                        yb, ps, bout_t[:, half * 512 : (half + 1) * 512]
                    )
                    nc.sync.dma_start(
                        out=y[qt_i * 128 : (qt_i + 1) * 128, half * 512 : (half + 1) * 512],
                        in_=yb,
                    )


def build_nc():
    nc = bacc.Bacc("TRN2", target_bir_lowering=False, debug=False, num_devices=N_CORES)
    x = nc.dram_tensor("x", [SH, E], F32R, kind="ExternalInput").ap()
    wqkv = nc.dram_tensor("wqkv", [E, 3 * E], F32R, kind="ExternalInput").ap()
    bqkv = nc.dram_tensor("bqkv", [3 * E], F32R, kind="ExternalInput").ap()
    wout = nc.dram_tensor("wout", [E, E], F32R, kind="ExternalInput").ap()
    bout = nc.dram_tensor("bout", [E], F32, kind="ExternalInput").ap()
    y = nc.dram_tensor("y", [Q, E], F32, kind="ExternalOutput").ap()
    kloc = nc.dram_tensor("kloc", [NP, 128, SH], BF16).ap()
    kfulls = [nc.dram_tensor(f"kfull{i}", [2, 4, 128, SH], BF16).ap() for i in range(2)]
    vloc = nc.dram_tensor("vloc", [SH, E], BF16).ap()
    vfulls = [nc.dram_tensor(f"vfull{i}", [2, 256, E], BF16).ap() for i in range(4)]
    rscr = nc.dram_tensor("rscr", [16, 1024], F32).ap()
    rscr2 = nc.dram_tensor("rscr2", [16, 1024], F32).ap()
    with tile.TileContext(nc) as tc:
        _emit(tc, nc, x, wqkv, bqkv, wout, bout, y,
              kloc, kfulls, vloc, vfulls, rscr, rscr2)
    nc.compile()
    return nc


_NC = None


def _get_nc():
    global _NC
    if _NC is None:
        _NC = build_nc()
    return _NC


def make_in_maps(x, W_qkv, b_qkv, W_out, b_out):
    x = np.ascontiguousarray(np.asarray(x, dtype=np.float32))
    W_qkv = np.ascontiguousarray(np.asarray(W_qkv, dtype=np.float32))
    b_qkv = np.ascontiguousarray(np.asarray(b_qkv, dtype=np.float32))
    W_out = np.ascontiguousarray(np.asarray(W_out, dtype=np.float32))
    b_out = np.ascontiguousarray(np.asarray(b_out, dtype=np.float32))
    in_maps = []
    for c in range(N_CORES):
        b, hq = c // 2, c % 2
        xb = np.ascontiguousarray(x[b, hq * SH : (hq + 1) * SH])
        in_maps.append(
            {"x": xb, "wqkv": W_qkv, "bqkv": b_qkv, "wout": W_out, "bout": b_out}
        )
    return in_maps


def assemble(results):
    out = np.empty((B, S, E), dtype=np.float32)
    for c in range(N_CORES):
        b, hq = c // 2, c % 2
        out[b, hq * 1024 : (hq + 1) * 1024, :] = results[c]["y"]
    return out


def kernel(x, W_qkv, b_qkv, W_out, b_out):
    nc = _get_nc()
    in_maps = make_in_maps(x, W_qkv, b_qkv, W_out, b_out)
    res = run_bass_kernel_spmd(nc, in_maps, list(range(N_CORES)))
    return assemble(res.results)


# revision 12
# speedup vs baseline: 1.1808x; 1.1808x over previous
"""Multi-head attention TRN2 kernel (B=4, S=2048, E=1024, H=16, D=64) on 8 cores.

Sharding: core c = (batch b = c//2, token-half hq = c%2). Each core receives
only its own 1024-token half of batch b's sequence (no rotation). K/V
projections are computed for the OWN half only and exchanged with the buddy
core via pairwise AllGather (replica groups {2b, 2b+1}), eliminating the
K/V-projection duplication. Keys stay in original order on both cores
(softmax over keys is order-invariant); queries are the own half, so each
core emits y rows for its own tokens.

Per-core dataflow:
  Phase A: x (f32) -> xs (bf16, converting DMA) -> xt via XBAR DMA transpose
    (no PE). V = x_own @ Wv + bv -> vloc -> AllGather -> vfull (chunked for
    overlap). K^T per pair for own tokens -> kloc -> AllGather -> kfull.
  Phase B: per head-pair p: Q^T JIT from own xt; kt loaded from kfull;
    scoresT [k, q] per k-tile as a row-tiled pair; exp split across scalar
    ACT (exact, exp(s/8 + ln lam)) and a custom DVE poly op; attnV with
    ones-augmented V stationary [k, 65] (row 64 = softmax denominator).
  Phase C: y = attn_outT.T @ W_out + b_out.
"""
from contextlib import ExitStack

import numpy as np

import concourse.bass as bass
import concourse.tile as tile
from concourse import bacc, mybir, dve_ops
from concourse.bass_utils import run_bass_kernel_spmd
from concourse.dve_spec import C0, C1, C2, Spec, Src0, sq
from concourse.tile_rust import add_dep_helper


def _ins(h):
    return getattr(h, "ins", h)


def _dep(after, before, why):
    add_dep_helper(_ins(after), _ins(before), reason=why)

F32R = mybir.dt.float32r
F32 = mybir.dt.float32
BF16 = mybir.dt.bfloat16
AF = mybir.ActivationFunctionType

B, S, E, H, D = 4, 2048, 1024, 16, 64
Q = 1024          # queries per core (own half)
SH = 1024         # own-half tokens for K/V projection
STO = 8           # own s-tiles
ET = 8            # e-tiles (contraction over E)
KT = 16           # k-tiles in attention (full S)
NP = 8            # head-pairs
N_CORES = 8
RG = [[0, 1], [2, 3], [4, 5], [6, 7]]

# lam*e^(s/8) ~ ((s - R)(s^2 + Bc*s + G))^4 for raw scores s in [-24, 24].
EXP_R = -5.36194375e+01
EXP_B = 4.77311991e+01
EXP_G = 3.76928874e+03
LN_LAM = 48.86652111696897   # scalar-ACT bias so both exp paths share lam
# k-tiles whose exp runs on the DVE custom op (rest: scalar ACT, exact)
DVE_KS = frozenset({1, 3, 5, 7, 9, 11, 13})


def _exp4_ref(in0, in1, s0, s1, imm2):
    pp = ((in0 - s0) * ((in0 * in0 + s1 * in0) + imm2)).astype(np.float32)
    y = (pp * pp).astype(np.float32)
    return (y * y).astype(np.float32)


def _register_exp4():
    for op in dve_ops.OPS:
        if op.name == "EXP4_ANT":
            return op
    x = Src0
    body = sq(sq((x - C0) * ((sq(x) + C1 * x) + C2)))
    op = dve_ops.DveOp(
        "EXP4_ANT",
        Spec(body=body, reference=_exp4_ref),
        subdim=False,
        uops_sha={"v3": "8097539a72e1c183"},
    )
    dve_ops.OPS.append(op)
    dve_ops.CUSTOM_DVE_SPECS[op.name] = op.spec
    dve_ops._SUB_OPCODE_FOR_NAME[op.name] = (
        dve_ops._CUSTOM_DVE_ROW_BASE + len(dve_ops.OPS) - 1
    )
    return op


EXP4 = _register_exp4()


def _bcast_dram(ap1d, n_part, n_free):
    """Broadcast a DRAM row across n_part partitions: [[0,n_part],[1,n_free]]."""
    return bass.AP(
        tensor=ap1d.tensor, offset=ap1d.offset, ap=[[0, n_part], [1, n_free]]
    )


def _emit(tc, nc, x, wqkv, bqkv, wout, bout, y,
          kloc, kfulls, vloc, vfulls, rscr, rscr2):
    with ExitStack() as ctx:
        xt_pool = ctx.enter_context(tc.tile_pool(name="xt", bufs=1))
        const = ctx.enter_context(tc.tile_pool(name="const", bufs=1))

        xt = xt_pool.tile([128, STO, ET, 128], BF16)

        wqb = const.tile([128, ET, E], BF16)
        wob = const.tile([128, ET, E], BF16)
        bqk_t = const.tile([128, 24], F32)
        nc.gpsimd.dma_start(
            out=bqk_t, in_=bqkv.rearrange("(j p) -> p j", p=128).bitcast(F32)
        )
        lnlam_t = const.tile([128, 1], F32)
        nc.vector.memset(lnlam_t, LN_LAM)

        # ---- phase A: xs -> xt (PE transpose), V proj own half, K proj own ----
        with (
            tc.tile_pool(name="fconst", bufs=1) as fconst,
            tc.tile_pool(name="xload", bufs=4) as xload,
            tc.tile_pool(name="vps", bufs=2, space="PSUM") as vps,
            tc.tile_pool(name="tps", bufs=2, space="PSUM") as tps,
            tc.tile_pool(name="vev", bufs=4) as vev,
        ):
            from concourse.masks import make_identity
            ident_f = fconst.tile([128, 128], F32)
            make_identity(nc, ident_f)
            ident = fconst.tile([128, 128], BF16)
            nc.vector.tensor_copy(ident, ident_f)
            wvb = fconst.tile([128, ET, E], BF16)
            wkb = fconst.tile([128, ET, E], BF16)
            for vh in range(2):
                nc.scalar.dma_start(
                    out=wvb[:, :, vh * 512 : (vh + 1) * 512],
                    in_=wqkv[:, 2 * E + vh * 512 : 2 * E + (vh + 1) * 512].rearrange(
                        "(t p) n -> p t n", p=128),
                )
                nc.sync.dma_start(
                    out=wkb[:, :, vh * 512 : (vh + 1) * 512],
                    in_=wqkv[:, E + vh * 512 : E + (vh + 1) * 512].rearrange(
                        "(t p) n -> p t n", p=128),
                )
            for vh in range(2):
                nc.scalar.dma_start(
                    out=wqb[:, :, vh * 512 : (vh + 1) * 512],
                    in_=wqkv[:, vh * 512 : (vh + 1) * 512].rearrange(
                        "(t p) n -> p t n", p=128),
                )
                nc.sync.dma_start(
                    out=wob[:, :, vh * 512 : (vh + 1) * 512],
                    in_=wout[:, vh * 512 : (vh + 1) * 512].rearrange(
                        "(t p) n -> p t n", p=128),
                )
            bv_t = fconst.tile([128, E], F32R)
            nc.gpsimd.dma_start(
                out=bv_t, in_=_bcast_dram(bqkv[2 * E : 2 * E + 1], 128, E)
            )

            def v_proj(st):
                ps = vps.tile([128, E], F32)
                for half in range(2):
                    for et in range(ET):
                        nc.tensor.matmul(
                            ps[:, half * 512 : (half + 1) * 512],
                            xt[:, st, et, :],
                            wvb[:, et, half * 512 : (half + 1) * 512],
                            start=(et == 0),
                            stop=(et == ET - 1),
                        )
                vb = vev.tile([128, E], BF16)
                nc.vector.tensor_add(vb, ps, bv_t)
                return nc.scalar.dma_start(
                    out=vloc[st * 128 : (st + 1) * 128, :], in_=vb)

            cc_v = [None] * 4
            vw = []
            for st in range(STO):
                xs = xload.tile([128, E], BF16, tag="xs")
                nc.sync.dma_start(out=xs, in_=x[st * 128 : (st + 1) * 128, :])
                for g in range(2):
                    ps = tps.tile([128, 4, 128], BF16)
                    for i in range(4):
                        e2 = g * 4 + i
                        nc.tensor.transpose(
                            ps[:, i, :], xs[:, e2 * 128 : (e2 + 1) * 128], ident
                        )
                    nc.vector.tensor_copy(xt[:, st, g * 4 : (g + 1) * 4, :], ps)
                vw.append(v_proj(st))
                if st % 2 == 1:
                    c = st // 2
                    cc = nc.gpsimd.collective_compute(
                        "AllGather",
                        mybir.AluOpType.bypass,
                        replica_groups=RG,
                        ins=[vloc[c * 256 : (c + 1) * 256, :]],
                        outs=[vfulls[c][:, :, :]],
                    )
                    for w in vw:
                        _dep(cc, w, "AG-V waits for vloc writes")
                    vw = []
                    cc_v[c] = cc

            def k_proj(p):
                ps = vps.tile([128, SH], F32)
                for half in range(2):
                    for et in range(ET):
                        nc.tensor.matmul(
                            ps[:, half * 512 : (half + 1) * 512],
                            wkb[:, et, p * 128 : (p + 1) * 128],
                            xt[:, half * 4 : (half + 1) * 4, et, :],
                            start=(et == 0),
                            stop=(et == ET - 1),
                        )
                kb = vev.tile([128, SH], BF16, tag="kb")
                nc.scalar.activation(
                    out=kb, in_=ps, func=AF.Identity,
                    bias=bqk_t[:, 8 + p : 9 + p], scale=1.0,
                )
                return nc.sync.dma_start(out=kloc[p], in_=kb)

            cc_k = [None] * 2
            kw = []
            for p in range(NP):
                kw.append(k_proj(p))
                if p % 4 == 3:
                    h4 = p // 4
                    cc = nc.gpsimd.collective_compute(
                        "AllGather",
                        mybir.AluOpType.bypass,
                        replica_groups=RG,
                        ins=[kloc[h4 * 4 : (h4 + 1) * 4]],
                        outs=[kfulls[h4][:, :, :, :]],
                    )
                    for w in kw:
                        _dep(cc, w, "AG-K waits for kloc writes")
                    kw = []
                    cc_k[h4] = cc

        ctx.cc_k, ctx.cc_v = cc_k, cc_v
        # ---- phase B: per-pair Q JIT + attention, software-pipelined ----
        aout_pool = ctx.enter_context(tc.tile_pool(name="aout", bufs=1))
        aout = aout_pool.tile([128, NP, Q], BF16)

        with (
            tc.tile_pool(name="qt", bufs=2) as qtp,
            tc.tile_pool(name="kt", bufs=2) as ktp,
            tc.tile_pool(name="vp", bufs=2) as vpp,
            tc.tile_pool(name="pt", bufs=4) as ptp,
            tc.tile_pool(name="ev", bufs=2) as evp,
            tc.tile_pool(name="qkps", bufs=1, space="PSUM") as qkps,
            tc.tile_pool(name="scps", bufs=2, space="PSUM") as scps,
            tc.tile_pool(name="accps", bufs=2, space="PSUM") as accps,
        ):

            def build_pair(p):
                """Allocate pair-p input tiles; return (tiles, emission thunks)."""
                qt_t = qtp.tile([128, Q], BF16)
                kt_t = ktp.tile([128, S], BF16)
                vp = vpp.tile([128, KT, 2, 65], BF16)
                th = []
                def kt_load(r, p=p, kt_t=kt_t):
                    d = nc.sync.dma_start(
                        out=kt_t[:, r * 1024 : (r + 1) * 1024],
                        in_=kfulls[p // 4][r, p % 4],
                    )
                    _dep(d, ctx.cc_k[p // 4], "kt load waits on AG-K")
                for r in range(2):
                    th.append(lambda r=r: kt_load(r))

                def vp_load(h, r, c, p=p, vp=vp):
                    d = nc.sync.dma_start(
                        out=vp[:, r * 8 + c * 2 : r * 8 + c * 2 + 2, h, 0:64],
                        in_=vfulls[c][
                            r, :, p * 128 + h * 64 : p * 128 + h * 64 + 64
                        ].rearrange("(t p2) d -> p2 t d", p2=128),
                    )
                    _dep(d, ctx.cc_v[c], "vp load waits on AG-V")
                for h in range(2):
                    for r in range(2):
                        for c in range(4):
                            th.append(lambda h=h, r=r, c=c: vp_load(h, r, c))
                th.append(lambda: nc.vector.memset(vp[:, :, :, 64:65], 1.0))

                # Q^T for own 1024 queries
                ps_box = []

                def alloc():
                    qk_ps = qkps.tile([128, 1024], F32, name="qk_ps", tag="qk")
                    ps_box.append(qk_ps)
                th.append(alloc)
                for half in range(2):
                    for et in range(ET):
                        th.append(lambda half=half, et=et: nc.tensor.matmul(
                            ps_box[0][:, half * 512 : (half + 1) * 512],
                            wqb[:, et, p * 128 : (p + 1) * 128],
                            xt[:, half * 4 : (half + 1) * 4, et, :],
                            start=(et == 0),
                            stop=(et == ET - 1),
                        ))
                th.append(lambda: nc.scalar.activation(
                    out=qt_t, in_=ps_box[0], func=AF.Identity,
                    bias=bqk_t[:, p : p + 1], scale=1.0,
                ))
                return {"qt": qt_t, "kt": kt_t, "vp": vp}, th

            cur, th0 = build_pair(0)
            for t in th0:
                t()
            ev_pending = []

            for p in range(NP):
                if p + 1 < NP:
                    nxt, pending = build_pair(p + 1)
                else:
                    nxt = None
                    pending = []
                pending = list(pending)
                qt_t, kt_t, vp = cur["qt"], cur["kt"], cur["vp"]
                for qh in range(2):
                    qsl = slice(qh * 512, (qh + 1) * 512)
                    acc0 = accps.tile([128, 512], F32, tag="acc")
                    acc1 = accps.tile([128, 512], F32, tag="acc")
                    pts = [None] * KT
                    for k in range(KT):
                        sc = scps.tile([128, 1024], F32, tag="sc")
                        nc.tensor.matmul(
                            sc[:, 0:512],
                            kt_t[0:64, k * 128 : (k + 1) * 128],
                            qt_t[0:64, qsl],
                            start=True, stop=True,
                        )
                        nc.tensor.matmul(
                            sc[:, 512:1024],
                            kt_t[64:128, k * 128 : (k + 1) * 128],
                            qt_t[64:128, qsl],
                            start=True, stop=True,
                        )
                        if k >= 1:
                            pt_p = pts[k - 1]
                            nc.tensor.matmul(
                                acc0[0:65, :], vp[:, k - 1, 0, :], pt_p[:, 0:512],
                                start=(k - 1 == 0), stop=(k - 1 == KT - 1),
                            )
                            nc.tensor.matmul(
                                acc1[0:65, :], vp[:, k - 1, 1, :], pt_p[:, 512:1024],
                                start=(k - 1 == 0), stop=(k - 1 == KT - 1),
                            )
                        pt_t = ptp.tile([128, 1024], BF16)
                        pts[k] = pt_t
                        if k in DVE_KS:
                            nc.vector._custom_dve(
                                EXP4, out=pt_t[:, :], in0=sc,
                                s0=EXP_R, s1=EXP_B, imm2=EXP_G,
                            )
                        else:
                            nc.scalar.activation(
                                out=pt_t, in_=sc, func=AF.Exp,
                                scale=0.125, bias=lnlam_t[:, 0:1],
                            )
                        for _ in range(2):
                            if pending:
                                pending.pop(0)()
                        if k in (4, 6) and ev_pending:
                            ev_pending.pop(0)()
                    nc.tensor.matmul(
                        acc0[0:65, :], vp[:, KT - 1, 0, :], pts[KT - 1][:, 0:512],
                        start=False, stop=True,
                    )
                    nc.tensor.matmul(
                        acc1[0:65, :], vp[:, KT - 1, 1, :], pts[KT - 1][:, 512:1024],
                        start=False, stop=True,
                    )
                    # eviction: fast psum release on scalar; the DVE pieces
                    # of the reciprocal dance are deferred into the next
                    # unit's loop so they never head-of-line block the exps.
                    ridx = p * 2 + qh
                    au0 = evp.tile([128, 512], F32, tag="au0")
                    nc.scalar.copy(au0[0:65, :], acc0[0:65, :])
                    au1 = evp.tile([128, 512], F32, tag="au1")
                    nc.scalar.copy(au1[0:65, :], acc1[0:65, :])
                    nc.gpsimd.dma_start(out=rscr[ridx : ridx + 1, 0:512], in_=au0[64:65, :])
                    nc.gpsimd.dma_start(out=rscr[ridx : ridx + 1, 512:1024], in_=au1[64:65, :])
                    rw = evp.tile([64, 16], F32, tag="rw")
                    nc.gpsimd.dma_start(
                        out=rw, in_=rscr[ridx : ridx + 1, :].rearrange("o (p f) -> (o p) f", p=64)
                    )
                    rwr = evp.tile([64, 16], F32, tag="rwr")

                    def dance1(rw=rw, rwr=rwr, ridx=ridx):
                        nc.vector.reciprocal(rwr, rw)
                        nc.gpsimd.dma_start(
                            out=rscr2[ridx : ridx + 1, :].rearrange("o (p f) -> (o p) f", p=64),
                            in_=rwr,
                        )

                    def dance2(au0=au0, au1=au1, ridx=ridx, p=p, qsl=qsl):
                        sc0 = evp.tile([64, 512], F32, name="sc0", tag="sc0")
                        nc.gpsimd.dma_start(out=sc0, in_=_bcast_dram(rscr2[ridx, 0:1], 64, 512))
                        sc1 = evp.tile([64, 512], F32, name="sc1", tag="sc1")
                        nc.gpsimd.dma_start(out=sc1, in_=_bcast_dram(rscr2[ridx, 512:513], 64, 512))
                        nc.vector.tensor_mul(aout[0:64, p, qsl], au0[0:64, :], sc0)
                        tmp1 = evp.tile([64, 512], BF16, name="tmp1", tag="tmp1")
                        nc.vector.tensor_mul(tmp1, au1[0:64, :], sc1)
                        nc.gpsimd.dma_start(out=aout[64:128, p, qsl], in_=tmp1)

                    ev_pending.append(dance1)
                    ev_pending.append(dance2)
                for t in pending:
                    t()
                cur = nxt
            for t in ev_pending:
                t()
            ev_pending.clear()

        # ---- phase C: y = attn_out @ W_out + b_out ----
        with (
            tc.tile_pool(name="yps", bufs=4, space="PSUM") as yps,
            tc.tile_pool(name="yev", bufs=3) as yev,
        ):
            bout_t = yev.tile([128, E], F32, tag="bout")
            nc.gpsimd.dma_start(out=bout_t, in_=_bcast_dram(bout[0:1], 128, E))
            for half in range(2):
                for qt_i in range(8):
                    ps = yps.tile([128, 512], F32)
                    for p8 in range(8):
                        nc.tensor.matmul(
                            ps,
                            aout[:, p8, qt_i * 128 : (qt_i + 1) * 128],
                            wob[:, p8, half * 512 : (half + 1) * 512],
                            start=(p8 == 0),
                            stop=(p8 == 7),
                        )
                    yb = yev.tile([128, 512], F32)
                    nc.vector.tensor_add(
                        yb, ps, bout_t[:, half * 512 : (half + 1) * 512]
                    )
                    nc.sync.dma_start(
                        out=y[qt_i * 128 : (qt_i + 1) * 128, half * 512 : (half + 1) * 512],
                        in_=yb,
                    )


def build_nc():
    nc = bacc.Bacc("TRN2", target_bir_lowering=False, debug=False, num_devices=N_CORES)
    x = nc.dram_tensor("x", [SH, E], BF16, kind="ExternalInput").ap()
    wqkv = nc.dram_tensor("wqkv", [E, 3 * E], BF16, kind="ExternalInput").ap()
    bqkv = nc.dram_tensor("bqkv", [3 * E], F32R, kind="ExternalInput").ap()
    wout = nc.dram_tensor("wout", [E, E], BF16, kind="ExternalInput").ap()
    bout = nc.dram_tensor("bout", [E], F32, kind="ExternalInput").ap()
    y = nc.dram_tensor("y", [Q, E], F32, kind="ExternalOutput").ap()
    kloc = nc.dram_tensor("kloc", [NP, 128, SH], BF16).ap()
    kfulls = [nc.dram_tensor(f"kfull{i}", [2, 4, 128, SH], BF16).ap() for i in range(2)]
    vloc = nc.dram_tensor("vloc", [SH, E], BF16).ap()
    vfulls = [nc.dram_tensor(f"vfull{i}", [2, 256, E], BF16).ap() for i in range(4)]
    rscr = nc.dram_tensor("rscr", [16, 1024], F32).ap()
    rscr2 = nc.dram_tensor("rscr2", [16, 1024], F32).ap()
    with tile.TileContext(nc) as tc:
        _emit(tc, nc, x, wqkv, bqkv, wout, bout, y,
              kloc, kfulls, vloc, vfulls, rscr, rscr2)
    nc.compile()
    return nc


_NC = None


def _get_nc():
    global _NC
    if _NC is None:
        _NC = build_nc()
    return _NC


def make_in_maps(x, W_qkv, b_qkv, W_out, b_out):
    import ml_dtypes
    x = np.asarray(x, dtype=np.float32).astype(ml_dtypes.bfloat16)
    W_qkv = np.ascontiguousarray(
        np.asarray(W_qkv, dtype=np.float32).astype(ml_dtypes.bfloat16))
    b_qkv = np.ascontiguousarray(np.asarray(b_qkv, dtype=np.float32))
    W_out = np.ascontiguousarray(
        np.asarray(W_out, dtype=np.float32).astype(ml_dtypes.bfloat16))
    b_out = np.ascontiguousarray(np.asarray(b_out, dtype=np.float32))
    in_maps = []
    for c in range(N_CORES):
        b, hq = c // 2, c % 2
        xb = np.ascontiguousarray(x[b, hq * SH : (hq + 1) * SH])
        in_maps.append(
            {"x": xb, "wqkv": W_qkv, "bqkv": b_qkv, "wout": W_out, "bout": b_out}
        )
    return in_maps


def assemble(results):
    out = np.empty((B, S, E), dtype=np.float32)
    for c in range(N_CORES):
        b, hq = c // 2, c % 2
        out[b, hq * 1024 : (hq + 1) * 1024, :] = results[c]["y"]
    return out


def kernel(x, W_qkv, b_qkv, W_out, b_out):
    nc = _get_nc()
    in_maps = make_in_maps(x, W_qkv, b_qkv, W_out, b_out)
    res = run_bass_kernel_spmd(nc, in_maps, list(range(N_CORES)))
    return assemble(res.results)


# revision 13
# speedup vs baseline: 1.2043x; 1.0199x over previous
"""Multi-head attention TRN2 kernel (B=4, S=2048, E=1024, H=16, D=64) on 8 cores.

Sharding: core c = (batch b = c//2, token-half hq = c%2). Each core receives
only its own 1024-token half of batch b's sequence (no rotation). K/V
projections are computed for the OWN half only and exchanged with the buddy
core via pairwise AllGather (replica groups {2b, 2b+1}), eliminating the
K/V-projection duplication. Keys stay in original order on both cores
(softmax over keys is order-invariant); queries are the own half, so each
core emits y rows for its own tokens.

Per-core dataflow:
  Phase A: x (f32) -> xs (bf16, converting DMA) -> xt via XBAR DMA transpose
    (no PE). V = x_own @ Wv + bv -> vloc -> AllGather -> vfull (chunked for
    overlap). K^T per pair for own tokens -> kloc -> AllGather -> kfull.
  Phase B: per head-pair p: Q^T JIT from own xt; kt loaded from kfull;
    scoresT [k, q] per k-tile as a row-tiled pair; exp split across scalar
    ACT (exact, exp(s/8 + ln lam)) and a custom DVE poly op; attnV with
    ones-augmented V stationary [k, 65] (row 64 = softmax denominator).
  Phase C: y = attn_outT.T @ W_out + b_out.
"""
from contextlib import ExitStack

import numpy as np

import concourse.bass as bass
import concourse.tile as tile
from concourse import bacc, mybir, dve_ops
from concourse.bass_utils import run_bass_kernel_spmd
from concourse.dve_spec import C0, C1, C2, Spec, Src0, sq
from concourse.tile_rust import add_dep_helper


def _ins(h):
    return getattr(h, "ins", h)


def _dep(after, before, why):
    add_dep_helper(_ins(after), _ins(before), reason=why)

F32R = mybir.dt.float32r
F32 = mybir.dt.float32
BF16 = mybir.dt.bfloat16
AF = mybir.ActivationFunctionType

B, S, E, H, D = 4, 2048, 1024, 16, 64
Q = 1024          # queries per core (own half)
SH = 1024         # own-half tokens for K/V projection
STO = 8           # own s-tiles
ET = 8            # e-tiles (contraction over E)
KT = 16           # k-tiles in attention (full S)
NP = 8            # head-pairs
N_CORES = 8
RG = [[0, 1], [2, 3], [4, 5], [6, 7]]

# lam*e^(s/8) ~ ((s - R)(s^2 + Bc*s + G))^4 for raw scores s in [-24, 24].
EXP_R = -5.36194375e+01
EXP_B = 4.77311991e+01
EXP_G = 3.76928874e+03
LN_LAM = 48.86652111696897   # scalar-ACT bias so both exp paths share lam
# k-tiles whose exp runs on the DVE custom op (rest: scalar ACT, exact)
DVE_KS = frozenset({1, 3, 5, 7, 9, 11, 13})


def _exp4_ref(in0, in1, s0, s1, imm2):
    pp = ((in0 - s0) * ((in0 * in0 + s1 * in0) + imm2)).astype(np.float32)
    y = (pp * pp).astype(np.float32)
    return (y * y).astype(np.float32)


def _register_exp4():
    for op in dve_ops.OPS:
        if op.name == "EXP4_ANT":
            return op
    x = Src0
    body = sq(sq((x - C0) * ((sq(x) + C1 * x) + C2)))
    op = dve_ops.DveOp(
        "EXP4_ANT",
        Spec(body=body, reference=_exp4_ref),
        subdim=False,
        uops_sha={"v3": "8097539a72e1c183"},
    )
    dve_ops.OPS.append(op)
    dve_ops.CUSTOM_DVE_SPECS[op.name] = op.spec
    dve_ops._SUB_OPCODE_FOR_NAME[op.name] = (
        dve_ops._CUSTOM_DVE_ROW_BASE + len(dve_ops.OPS) - 1
    )
    return op


EXP4 = _register_exp4()


def _bcast_dram(ap1d, n_part, n_free):
    """Broadcast a DRAM row across n_part partitions: [[0,n_part],[1,n_free]]."""
    return bass.AP(
        tensor=ap1d.tensor, offset=ap1d.offset, ap=[[0, n_part], [1, n_free]]
    )


def _emit(tc, nc, x, w_pre, bqkv, bout, y,
          kloc, kfulls, vloc, vfulls, rscr, rscr2):
    wq_p, wk_p, wv_p, wo_p = w_pre
    with ExitStack() as ctx:
        xt_pool = ctx.enter_context(tc.tile_pool(name="xt", bufs=1))
        const = ctx.enter_context(tc.tile_pool(name="const", bufs=1))

        xt = xt_pool.tile([128, STO, ET, 128], BF16)

        wqb = const.tile([128, ET, E], BF16)
        wob = const.tile([128, ET, E], BF16)
        bqk_t = const.tile([128, 24], F32)
        nc.gpsimd.dma_start(
            out=bqk_t, in_=bqkv.rearrange("(j p) -> p j", p=128).bitcast(F32)
        )
        lnlam_t = const.tile([128, 1], F32)
        nc.vector.memset(lnlam_t, LN_LAM)

        # ---- phase A: xs -> xt (PE transpose), V proj own half, K proj own ----
        with (
            tc.tile_pool(name="fconst", bufs=1) as fconst,
            tc.tile_pool(name="xload", bufs=4) as xload,
            tc.tile_pool(name="vps", bufs=2, space="PSUM") as vps,
            tc.tile_pool(name="tps", bufs=2, space="PSUM") as tps,
            tc.tile_pool(name="vev", bufs=4) as vev,
        ):
            from concourse.masks import make_identity
            ident_f = fconst.tile([128, 128], F32)
            make_identity(nc, ident_f)
            ident = fconst.tile([128, 128], BF16)
            nc.vector.tensor_copy(ident, ident_f)
            wvb = fconst.tile([128, ET, E], BF16)
            wkb = fconst.tile([128, ET, E], BF16)
            nc.scalar.dma_start(out=wvb, in_=wv_p[:, :, :])
            nc.gpsimd.dma_start(out=wkb, in_=wk_p[:, :, :])
            nc.scalar.dma_start(out=wqb, in_=wq_p[:, :, :])
            nc.gpsimd.dma_start(out=wob, in_=wo_p[:, :, :])
            bv_t = fconst.tile([128, E], F32R)
            nc.gpsimd.dma_start(
                out=bv_t, in_=_bcast_dram(bqkv[2 * E : 2 * E + 1], 128, E)
            )

            def v_proj(st):
                ps = vps.tile([128, E], F32)
                for half in range(2):
                    for et in range(ET):
                        nc.tensor.matmul(
                            ps[:, half * 512 : (half + 1) * 512],
                            xt[:, st, et, :],
                            wvb[:, et, half * 512 : (half + 1) * 512],
                            start=(et == 0),
                            stop=(et == ET - 1),
                        )
                vb = vev.tile([128, E], BF16)
                nc.vector.tensor_add(vb, ps, bv_t)
                return nc.scalar.dma_start(
                    out=vloc[st * 128 : (st + 1) * 128, :], in_=vb)

            cc_v = [None] * 4
            vw = []
            for st in range(STO):
                xs = xload.tile([128, E], BF16, tag="xs")
                nc.sync.dma_start(out=xs, in_=x[st * 128 : (st + 1) * 128, :])
                for g in range(2):
                    ps = tps.tile([128, 4, 128], BF16)
                    for i in range(4):
                        e2 = g * 4 + i
                        nc.tensor.transpose(
                            ps[:, i, :], xs[:, e2 * 128 : (e2 + 1) * 128], ident
                        )
                    nc.vector.tensor_copy(xt[:, st, g * 4 : (g + 1) * 4, :], ps)
                vw.append(v_proj(st))
                if st % 2 == 1:
                    c = st // 2
                    cc = nc.gpsimd.collective_compute(
                        "AllGather",
                        mybir.AluOpType.bypass,
                        replica_groups=RG,
                        ins=[vloc[c * 256 : (c + 1) * 256, :]],
                        outs=[vfulls[c][:, :, :]],
                    )
                    for w in vw:
                        _dep(cc, w, "AG-V waits for vloc writes")
                    vw = []
                    cc_v[c] = cc

            def k_proj(p):
                ps = vps.tile([128, SH], F32)
                for half in range(2):
                    for et in range(ET):
                        nc.tensor.matmul(
                            ps[:, half * 512 : (half + 1) * 512],
                            wkb[:, et, p * 128 : (p + 1) * 128],
                            xt[:, half * 4 : (half + 1) * 4, et, :],
                            start=(et == 0),
                            stop=(et == ET - 1),
                        )
                kb = vev.tile([128, SH], BF16, tag="kb")
                nc.scalar.activation(
                    out=kb, in_=ps, func=AF.Identity,
                    bias=bqk_t[:, 8 + p : 9 + p], scale=1.0,
                )
                return nc.sync.dma_start(out=kloc[p], in_=kb)

            cc_k = [None] * NP
            for p in range(NP):
                w = k_proj(p)
                cc = nc.gpsimd.collective_compute(
                    "AllGather",
                    mybir.AluOpType.bypass,
                    replica_groups=RG,
                    ins=[kloc[p]],
                    outs=[kfulls[p][:, :, :]],
                )
                _dep(cc, w, "AG-K waits for kloc write")
                cc_k[p] = cc

        ctx.cc_k, ctx.cc_v = cc_k, cc_v
        # ---- phase B: per-pair Q JIT + attention, software-pipelined ----
        aout_pool = ctx.enter_context(tc.tile_pool(name="aout", bufs=1))
        aout = aout_pool.tile([128, NP, Q], BF16)

        with (
            tc.tile_pool(name="qt", bufs=2) as qtp,
            tc.tile_pool(name="kt", bufs=2) as ktp,
            tc.tile_pool(name="vp", bufs=2) as vpp,
            tc.tile_pool(name="pt", bufs=4) as ptp,
            tc.tile_pool(name="ev", bufs=2) as evp,
            tc.tile_pool(name="qkps", bufs=1, space="PSUM") as qkps,
            tc.tile_pool(name="scps", bufs=2, space="PSUM") as scps,
            tc.tile_pool(name="accps", bufs=2, space="PSUM") as accps,
        ):

            def build_pair(p):
                """Allocate pair-p input tiles; return (tiles, emission thunks)."""
                qt_t = qtp.tile([128, Q], BF16)
                kt_t = ktp.tile([128, S], BF16)
                vp = vpp.tile([128, KT, 2, 65], BF16)
                th = []
                def kt_load(r, p=p, kt_t=kt_t):
                    d = nc.sync.dma_start(
                        out=kt_t[:, r * 1024 : (r + 1) * 1024],
                        in_=kfulls[p][r],
                    )
                    _dep(d, ctx.cc_k[p], "kt load waits on AG-K")
                for r in range(2):
                    th.append(lambda r=r: kt_load(r))

                def vp_load(h, r, c, p=p, vp=vp):
                    d = nc.sync.dma_start(
                        out=vp[:, r * 8 + c * 2 : r * 8 + c * 2 + 2, h, 0:64],
                        in_=vfulls[c][
                            r, :, p * 128 + h * 64 : p * 128 + h * 64 + 64
                        ].rearrange("(t p2) d -> p2 t d", p2=128),
                    )
                    _dep(d, ctx.cc_v[c], "vp load waits on AG-V")
                for h in range(2):
                    for r in range(2):
                        for c in range(4):
                            th.append(lambda h=h, r=r, c=c: vp_load(h, r, c))
                th.append(lambda: nc.vector.memset(vp[:, :, :, 64:65], 1.0))

                # Q^T for own 1024 queries
                ps_box = []

                def alloc():
                    qk_ps = qkps.tile([128, 1024], F32, name="qk_ps", tag="qk")
                    ps_box.append(qk_ps)
                th.append(alloc)
                for half in range(2):
                    for et in range(ET):
                        th.append(lambda half=half, et=et: nc.tensor.matmul(
                            ps_box[0][:, half * 512 : (half + 1) * 512],
                            wqb[:, et, p * 128 : (p + 1) * 128],
                            xt[:, half * 4 : (half + 1) * 4, et, :],
                            start=(et == 0),
                            stop=(et == ET - 1),
                        ))
                th.append(lambda: nc.scalar.activation(
                    out=qt_t, in_=ps_box[0], func=AF.Identity,
                    bias=bqk_t[:, p : p + 1], scale=1.0,
                ))
                return {"qt": qt_t, "kt": kt_t, "vp": vp}, th

            cur, th0 = build_pair(0)
            for t in th0:
                t()
            ev_pending = []

            for p in range(NP):
                if p + 1 < NP:
                    nxt, pending = build_pair(p + 1)
                else:
                    nxt = None
                    pending = []
                pending = list(pending)
                qt_t, kt_t, vp = cur["qt"], cur["kt"], cur["vp"]
                for qh in range(2):
                    qsl = slice(qh * 512, (qh + 1) * 512)
                    acc0 = accps.tile([128, 512], F32, tag="acc")
                    acc1 = accps.tile([128, 512], F32, tag="acc")
                    pts = [None] * KT
                    for k in range(KT):
                        sc = scps.tile([128, 1024], F32, tag="sc")
                        nc.tensor.matmul(
                            sc[:, 0:512],
                            kt_t[0:64, k * 128 : (k + 1) * 128],
                            qt_t[0:64, qsl],
                            start=True, stop=True,
                        )
                        nc.tensor.matmul(
                            sc[:, 512:1024],
                            kt_t[64:128, k * 128 : (k + 1) * 128],
                            qt_t[64:128, qsl],
                            start=True, stop=True,
                        )
                        if k >= 1:
                            pt_p = pts[k - 1]
                            nc.tensor.matmul(
                                acc0[0:65, :], vp[:, k - 1, 0, :], pt_p[:, 0:512],
                                start=(k - 1 == 0), stop=(k - 1 == KT - 1),
                            )
                            nc.tensor.matmul(
                                acc1[0:65, :], vp[:, k - 1, 1, :], pt_p[:, 512:1024],
                                start=(k - 1 == 0), stop=(k - 1 == KT - 1),
                            )
                        pt_t = ptp.tile([128, 1024], BF16)
                        pts[k] = pt_t
                        if k in DVE_KS:
                            nc.vector._custom_dve(
                                EXP4, out=pt_t[:, :], in0=sc,
                                s0=EXP_R, s1=EXP_B, imm2=EXP_G,
                            )
                        else:
                            nc.scalar.activation(
                                out=pt_t, in_=sc, func=AF.Exp,
                                scale=0.125, bias=lnlam_t[:, 0:1],
                            )
                        for _ in range(2):
                            if pending:
                                pending.pop(0)()
                        if k in (4, 6) and ev_pending:
                            ev_pending.pop(0)()
                    nc.tensor.matmul(
                        acc0[0:65, :], vp[:, KT - 1, 0, :], pts[KT - 1][:, 0:512],
                        start=False, stop=True,
                    )
                    nc.tensor.matmul(
                        acc1[0:65, :], vp[:, KT - 1, 1, :], pts[KT - 1][:, 512:1024],
                        start=False, stop=True,
                    )
                    # eviction: fast psum release on scalar; the DVE pieces
                    # of the reciprocal dance are deferred into the next
                    # unit's loop so they never head-of-line block the exps.
                    ridx = p * 2 + qh
                    au0 = evp.tile([128, 512], F32, tag="au0")
                    nc.scalar.copy(au0[0:65, :], acc0[0:65, :])
                    au1 = evp.tile([128, 512], F32, tag="au1")
                    nc.scalar.copy(au1[0:65, :], acc1[0:65, :])
                    nc.gpsimd.dma_start(out=rscr[ridx : ridx + 1, 0:512], in_=au0[64:65, :])
                    nc.gpsimd.dma_start(out=rscr[ridx : ridx + 1, 512:1024], in_=au1[64:65, :])
                    rw = evp.tile([64, 16], F32, tag="rw")
                    nc.gpsimd.dma_start(
                        out=rw, in_=rscr[ridx : ridx + 1, :].rearrange("o (p f) -> (o p) f", p=64)
                    )
                    rwr = evp.tile([64, 16], F32, tag="rwr")

                    def dance1(rw=rw, rwr=rwr, ridx=ridx):
                        nc.vector.reciprocal(rwr, rw)
                        nc.gpsimd.dma_start(
                            out=rscr2[ridx : ridx + 1, :].rearrange("o (p f) -> (o p) f", p=64),
                            in_=rwr,
                        )

                    def dance2(au0=au0, au1=au1, ridx=ridx, p=p, qsl=qsl):
                        sc0 = evp.tile([64, 512], F32, name="sc0", tag="sc0")
                        nc.gpsimd.dma_start(out=sc0, in_=_bcast_dram(rscr2[ridx, 0:1], 64, 512))
                        sc1 = evp.tile([64, 512], F32, name="sc1", tag="sc1")
                        nc.gpsimd.dma_start(out=sc1, in_=_bcast_dram(rscr2[ridx, 512:513], 64, 512))
                        nc.vector.tensor_mul(aout[0:64, p, qsl], au0[0:64, :], sc0)
                        tmp1 = evp.tile([64, 512], BF16, name="tmp1", tag="tmp1")
                        nc.vector.tensor_mul(tmp1, au1[0:64, :], sc1)
                        nc.gpsimd.dma_start(out=aout[64:128, p, qsl], in_=tmp1)

                    ev_pending.append(dance1)
                    ev_pending.append(dance2)
                for t in pending:
                    t()
                cur = nxt
            for t in ev_pending:
                t()
            ev_pending.clear()

        # ---- phase C: y = attn_out @ W_out + b_out ----
        with (
            tc.tile_pool(name="yps", bufs=4, space="PSUM") as yps,
            tc.tile_pool(name="yev", bufs=3) as yev,
        ):
            bout_t = yev.tile([128, E], F32, tag="bout")
            nc.gpsimd.dma_start(out=bout_t, in_=_bcast_dram(bout[0:1], 128, E))
            for half in range(2):
                for qt_i in range(8):
                    ps = yps.tile([128, 512], F32)
                    for p8 in range(8):
                        nc.tensor.matmul(
                            ps,
                            aout[:, p8, qt_i * 128 : (qt_i + 1) * 128],
                            wob[:, p8, half * 512 : (half + 1) * 512],
                            start=(p8 == 0),
                            stop=(p8 == 7),
                        )
                    yb = yev.tile([128, 512], F32)
                    nc.vector.tensor_add(
                        yb, ps, bout_t[:, half * 512 : (half + 1) * 512]
                    )
                    nc.sync.dma_start(
                        out=y[qt_i * 128 : (qt_i + 1) * 128, half * 512 : (half + 1) * 512],
                        in_=yb,
                    )


def build_nc():
    nc = bacc.Bacc("TRN2", target_bir_lowering=False, debug=False, num_devices=N_CORES)
    x = nc.dram_tensor("x", [SH, E], BF16, kind="ExternalInput").ap()
    wq_p = nc.dram_tensor("wq_p", [128, ET, E], BF16, kind="ExternalInput").ap()
    wk_p = nc.dram_tensor("wk_p", [128, ET, E], BF16, kind="ExternalInput").ap()
    wv_p = nc.dram_tensor("wv_p", [128, ET, E], BF16, kind="ExternalInput").ap()
    wo_p = nc.dram_tensor("wo_p", [128, ET, E], BF16, kind="ExternalInput").ap()
    bqkv = nc.dram_tensor("bqkv", [3 * E], F32R, kind="ExternalInput").ap()
    bout = nc.dram_tensor("bout", [E], F32, kind="ExternalInput").ap()
    y = nc.dram_tensor("y", [Q, E], F32, kind="ExternalOutput").ap()
    kloc = nc.dram_tensor("kloc", [NP, 128, SH], BF16).ap()
    kfulls = [nc.dram_tensor(f"kfull{i}", [2, 128, SH], BF16).ap() for i in range(NP)]
    vloc = nc.dram_tensor("vloc", [SH, E], BF16).ap()
    vfulls = [nc.dram_tensor(f"vfull{i}", [2, 256, E], BF16).ap() for i in range(4)]
    rscr = nc.dram_tensor("rscr", [16, 1024], F32).ap()
    rscr2 = nc.dram_tensor("rscr2", [16, 1024], F32).ap()
    with tile.TileContext(nc) as tc:
        _emit(tc, nc, x, (wq_p, wk_p, wv_p, wo_p), bqkv, bout, y,
              kloc, kfulls, vloc, vfulls, rscr, rscr2)
    nc.compile()
    return nc


_NC = None


def _get_nc():
    global _NC
    if _NC is None:
        _NC = build_nc()
    return _NC


def _prearrange(w):
    # [E, E] -> [128, ET, E]: partition p, tile et holds row et*128+p
    return np.ascontiguousarray(w.reshape(ET, 128, E).transpose(1, 0, 2))


def make_in_maps(x, W_qkv, b_qkv, W_out, b_out):
    import ml_dtypes
    x = np.asarray(x, dtype=np.float32).astype(ml_dtypes.bfloat16)
    W_qkv = np.asarray(W_qkv, dtype=np.float32).astype(ml_dtypes.bfloat16)
    b_qkv = np.ascontiguousarray(np.asarray(b_qkv, dtype=np.float32))
    W_out = np.asarray(W_out, dtype=np.float32).astype(ml_dtypes.bfloat16)
    b_out = np.ascontiguousarray(np.asarray(b_out, dtype=np.float32))
    wq_p = _prearrange(W_qkv[:, :E])
    wk_p = _prearrange(W_qkv[:, E : 2 * E])
    wv_p = _prearrange(W_qkv[:, 2 * E :])
    wo_p = _prearrange(W_out)
    in_maps = []
    for c in range(N_CORES):
        b, hq = c // 2, c % 2
        xb = np.ascontiguousarray(x[b, hq * SH : (hq + 1) * SH])
        in_maps.append(
            {"x": xb, "wq_p": wq_p, "wk_p": wk_p, "wv_p": wv_p, "wo_p": wo_p,
             "bqkv": b_qkv, "bout": b_out}
        )
    return in_maps


def assemble(results):
    out = np.empty((B, S, E), dtype=np.float32)
    for c in range(N_CORES):
        b, hq = c // 2, c % 2
        out[b, hq * 1024 : (hq + 1) * 1024, :] = results[c]["y"]
    return out


def kernel(x, W_qkv, b_qkv, W_out, b_out):
    nc = _get_nc()
    in_maps = make_in_maps(x, W_qkv, b_qkv, W_out, b_out)
    res = run_bass_kernel_spmd(nc, in_maps, list(range(N_CORES)))
    return assemble(res.results)
